# revision 8
# baseline (speedup 1.0000x reference)
"""AttentionPairBias TRN2 kernel — 8-core SPMD, query-row sharding.

Layout strategy (per core, q-block = 128 rows):
  - host folds LN(s) scale/bias and 1/sqrt(HD) into the projection weights,
    and folds the z-LN mean term into Wz:  zb = alpha * (z @ W'')  where
    W'' = diag(z_norm_w)@Wz - ones*colsum(.)/DZ  (the per-(q,h) bias row
    b' = z_norm_b@Wz is softmax-invariant and dropped).
  - host pre-transposes each core's z shard to zT [c=128, k=1024, q=128] so
    the PE contracts over c with z planes as the stationary operand.
  - device: Sum_c z comes from an appended ones column in the stationary's
    moving weights; Sum_c z^2 from a squared copy of each z plane vs a ones
    vector; alpha = rsqrt(var+eps) applied as a broadcast multiply.
  - scores[q,k] per head = QK^T (f32r) + zb added via an identity-stationary
    matmul accumulating into the same PSUM bank; exp on ACT with row-sum
    accumulation; probabilities normalized on DVE; o^T accumulated per head
    via PE with V as stationary; output rows = (o*g) @ Wo; host concatenates
    the 8 row-shards.
"""
import sys, os
sys.path.insert(0, "/opt/trn_rl_repo")
import numpy as np

import concourse.bass as bass
import concourse.bacc as bacc
import concourse.mybir as mybir
import concourse.tile as tile
from concourse.bass_utils import run_bass_kernel_spmd

F32 = mybir.dt.float32
F32R = mybir.dt.float32r
AF = mybir.ActivationFunctionType
OP = mybir.AluOpType

B, N, H, HD, D, DZ = 1, 1024, 16, 32, 512, 128
NC = 8
NQ = N // NC          # 128 q rows per core
KC = 32               # k's per DMA chunk
EPS = 1e-5

_CACHED = None


def _build():
    nc = bacc.Bacc(None, target_bir_lowering=False)

    s_d = nc.dram_tensor("s_full", [N, D], F32, kind="ExternalInput")
    sq_d = nc.dram_tensor("s_q", [NQ, D], F32, kind="ExternalInput")
    zT_d = nc.dram_tensor("zT", [DZ, N, NQ], F32R, kind="ExternalInput")
    wq_d = nc.dram_tensor("Wq", [D, D], F32R, kind="ExternalInput")
    wk_d = nc.dram_tensor("Wk", [D, D], F32R, kind="ExternalInput")
    wv_d = nc.dram_tensor("Wv", [D, D], F32R, kind="ExternalInput")
    wg_d = nc.dram_tensor("Wg", [D, D], F32R, kind="ExternalInput")
    wo_d = nc.dram_tensor("Wo", [D, D], F32R, kind="ExternalInput")
    bq_d = nc.dram_tensor("bq", [D], F32, kind="ExternalInput")
    bk_d = nc.dram_tensor("bk", [D], F32, kind="ExternalInput")
    bv_d = nc.dram_tensor("bv", [D], F32, kind="ExternalInput")
    bg_d = nc.dram_tensor("bg", [D], F32, kind="ExternalInput")
    wext_d = nc.dram_tensor("Wext", [DZ, 18], F32R, kind="ExternalInput")
    id_d = nc.dram_tensor("ident", [128, 128], F32R, kind="ExternalInput")
    out_d = nc.dram_tensor("out", [NQ, D], F32, kind="ExternalOutput")

    with tile.TileContext(nc) as tc:
        with tc.tile_pool(name="const", bufs=1) as cpool, \
             tc.tile_pool(name="persist", bufs=1) as pp:
            ident = cpool.tile([128, 128], F32R)
            nc.sync.dma_start(out=ident, in_=id_d[:, :])
            wext = cpool.tile([DZ, 18], F32R)
            nc.sync.dma_start(out=wext, in_=wext_d[:, :])
            eps_t = cpool.tile([128, 1], F32)
            nc.vector.memset(eps_t, EPS)
            bq_t = cpool.tile([128, 4], F32)
            nc.sync.dma_start(out=bq_t, in_=bq_d[:].rearrange("(b p) -> p b", p=128))
            bk_t = cpool.tile([128, 4], F32)
            nc.sync.dma_start(out=bk_t, in_=bk_d[:].rearrange("(b p) -> p b", p=128))
            bg_rep = cpool.tile([128, D], F32)
            bg_ap = bg_d[:]
            nc.gpsimd.dma_start(
                out=bg_rep,
                in_=bass.AP(tensor=bg_ap.tensor, offset=bg_ap.offset,
                            ap=[[0, 128], [1, D]]),
            )
            bv_rep = cpool.tile([128, D], F32)
            bv_ap = bv_d[:]
            nc.gpsimd.dma_start(
                out=bv_rep,
                in_=bass.AP(tensor=bv_ap.tensor, offset=bv_ap.offset,
                            ap=[[0, 128], [1, D]]),
            )

            # ---------- persistent activation storage ----------
            slnT = [pp.tile([128, N], F32R, name=f"slnT{j}") for j in range(4)]
            sqT = pp.tile([128, 4, 128], F32R)        # (d%128, dtile, q)
            KT = [pp.tile([128, N], F32R, name=f"KT{b}") for b in range(4)]
            Vt = [pp.tile([128, D], F32R, name=f"V{t}") for t in range(8)]
            QT = [pp.tile([128, 128], F32R, name=f"QT{b}") for b in range(4)]
            G_sb = pp.tile([128, D], F32, name="G_sb")
            zb = pp.tile([128, N, H], F32R)           # (q, k, h)
            rowsums = pp.tile([128, H], F32)

            # ================= phase A: s path =================
            with tc.tile_pool(name="sA", bufs=3) as ap_, \
                 tc.tile_pool(name="wA", bufs=1) as wp, \
                 tc.tile_pool(name="psA", bufs=2, space="PSUM") as psA:
                wk = [wp.tile([128, D], F32R, name=f"wk{i}") for i in range(4)]
                wv = [wp.tile([128, D], F32R, name=f"wv{i}") for i in range(4)]
                wq = [wp.tile([128, D], F32R, name=f"wq{i}") for i in range(4)]
                wg = [wp.tile([128, D], F32R, name=f"wg{i}") for i in range(4)]
                for i in range(4):
                    sl = slice(i * 128, (i + 1) * 128)
                    nc.sync.dma_start(out=wk[i], in_=wk_d[sl, :])
                    nc.sync.dma_start(out=wv[i], in_=wv_d[sl, :])
                    nc.sync.dma_start(out=wq[i], in_=wq_d[sl, :])
                    nc.sync.dma_start(out=wg[i], in_=wg_d[sl, :])

                def layernorm_tile(src_ap, tag):
                    st = ap_.tile([128, D], F32, tag="st", name=f"st{tag}")
                    nc.sync.dma_start(out=st, in_=src_ap)
                    stats = ap_.tile([128, 6], F32, tag="stats", name=f"stats{tag}")
                    nc.vector.bn_stats(out=stats, in_=st)
                    mv = ap_.tile([128, 2], F32, tag="mv", name=f"mv{tag}")
                    nc.vector.bn_aggr(out=mv, in_=stats)
                    std = ap_.tile([128, 1], F32, tag="std", name=f"std{tag}")
                    nc.scalar.activation(out=std, in_=mv[:, 1:2], func=AF.Sqrt,
                                         bias=eps_t, scale=1.0)
                    rst = ap_.tile([128, 1], F32, tag="rst", name=f"rst{tag}")
                    nc.vector.reciprocal(out=rst, in_=std)
                    sln = ap_.tile([128, D], F32R, tag="sln", name=f"sln{tag}")
                    nc.vector.scalar_tensor_tensor(
                        out=sln, in0=st, scalar=mv[:, 0:1],
                        in1=rst.to_broadcast((128, D)),
                        op0=OP.subtract, op1=OP.mult)
                    return sln

                # full-s LN + transpose into slnT
                for t in range(8):
                    sln = layernorm_tile(s_d[t * 128:(t + 1) * 128, :], f"s{t}")
                    ps = psA.tile([128, D], F32R, tag="trA")
                    for j in range(4):
                        nc.tensor.transpose(ps[:, j * 128:(j + 1) * 128],
                                            sln[:, j * 128:(j + 1) * 128], ident)
                    for j in range(4):
                        nc.scalar.copy(slnT[j][:, t * 128:(t + 1) * 128],
                                       ps[:, j * 128:(j + 1) * 128])
                # q-block LN + transpose into sqT
                slnq = layernorm_tile(sq_d[:, :], "q")
                psq = psA.tile([128, D], F32R, tag="trA")
                for j in range(4):
                    nc.tensor.transpose(psq[:, j * 128:(j + 1) * 128],
                                        slnq[:, j * 128:(j + 1) * 128], ident)
                for j in range(4):
                    nc.scalar.copy(sqT[:, j, :], psq[:, j * 128:(j + 1) * 128])

                # KT[b] = (sln @ Wk + bk)^T  -> [hd(128b), tok]
                for b in range(4):
                    bs = slice(b * 128, (b + 1) * 128)
                    for half in range(2):
                        hs = slice(half * 512, (half + 1) * 512)
                        ps = psA.tile([128, 512], F32, tag="mmA")
                        for dt_ in range(4):
                            nc.tensor.matmul(ps, wk[dt_][:, bs], slnT[dt_][:, hs],
                                             start=(dt_ == 0), stop=(dt_ == 3))
                        nc.scalar.activation(out=KT[b][:, hs], in_=ps,
                                             func=AF.Identity, bias=bk_t[:, b:b + 1],
                                             scale=1.0)
                # V[t] = sln @ Wv + bv  (natural [tok, hd])
                for t in range(8):
                    ts = slice(t * 128, (t + 1) * 128)
                    ps = psA.tile([128, 512], F32, tag="mmA")
                    for dt_ in range(4):
                        nc.tensor.matmul(ps, slnT[dt_][:, ts], wv[dt_],
                                         start=(dt_ == 0), stop=(dt_ == 3))
                    nc.vector.tensor_add(Vt[t], ps, bv_rep)
                # QT[b] from the q-block
                for b in range(4):
                    bs = slice(b * 128, (b + 1) * 128)
                    psqt = psA.tile([128, 128], F32, tag="qgA")
                    for dt_ in range(4):
                        nc.tensor.matmul(psqt, wq[dt_][:, bs], sqT[:, dt_, :],
                                         start=(dt_ == 0), stop=(dt_ == 3))
                    nc.scalar.activation(out=QT[b], in_=psqt, func=AF.Identity,
                                         bias=bq_t[:, b:b + 1], scale=1.0)
                # G natural [q, D]
                psg = psA.tile([128, D], F32, tag="mmA")
                for dt_ in range(4):
                    nc.tensor.matmul(psg, sqT[:, dt_, :], wg[dt_],
                                     start=(dt_ == 0), stop=(dt_ == 3))
                gsum = ap_.tile([128, D], F32, tag="st", name="gsum")
                nc.vector.tensor_add(gsum, psg, bg_rep)
                nc.scalar.activation(out=G_sb, in_=gsum, func=AF.Sigmoid,
                                     bias=0.0, scale=1.0)

            # ================= phase B: z path =================
            with tc.tile_pool(name="zB", bufs=2) as zp, \
                 tc.tile_pool(name="aB", bufs=2) as abuf, \
                 tc.tile_pool(name="psB", bufs=2, space="PSUM") as psB:
                for ci in range(N // KC):
                    zt = zp.tile([128, KC, 128], F32R, tag="zt")
                    nc.sync.dma_start(
                        out=zt, in_=zT_d[:, ci * KC:(ci + 1) * KC, :])
                    z2 = zp.tile([128, KC, 128], F32R, tag="z2")
                    flat_in = zt.rearrange("c k q -> c (k q)")
                    flat_out = z2.rearrange("c k q -> c (k q)")
                    hfl = KC * 128 // 2
                    nc.scalar.square(flat_out[:, 0:hfl], flat_in[:, 0:hfl])
                    nc.vector.tensor_mul(flat_out[:, hfl:], flat_in[:, hfl:],
                                         flat_in[:, hfl:])
                    for half in range(2):
                        ps = psB.tile([128, 512], F32, tag="zps")
                        for j in range(16):
                            kk = half * 16 + j
                            nc.tensor.matmul(ps[:, j * 18:(j + 1) * 18],
                                             zt[:, kk, :], wext,
                                             start=True, stop=True)
                            nc.tensor.matmul(ps[:, 288 + 2 * j:290 + 2 * j],
                                             z2[:, kk, :], wext[:, 16:18],
                                             start=True, stop=True)
                        raw3 = ps[:, 0:288].rearrange("p (k h) -> p k h", h=18)
                        mus = abuf.tile([128, 16], F32, tag="mus")
                        nc.scalar.mul(mus, raw3[:, :, 16], 1.0 / DZ)
                        tss = abuf.tile([128, 16], F32, tag="tss")
                        nc.scalar.mul(tss, ps[:, 288:320].rearrange("p (k two) -> p k two", two=2)[:, :, 0], 1.0 / DZ)
                        mu2 = abuf.tile([128, 16], F32, tag="mu2")
                        nc.vector.tensor_mul(mu2, mus, mus)
                        var = abuf.tile([128, 16], F32, tag="var")
                        nc.vector.tensor_sub(var, tss, mu2)
                        stdz = abuf.tile([128, 16], F32, tag="stdz")
                        nc.scalar.activation(out=stdz, in_=var, func=AF.Sqrt,
                                             bias=eps_t, scale=1.0)
                        alpha = abuf.tile([128, 16], F32, tag="alpha")
                        nc.vector.reciprocal(alpha, stdz)
                        alpha_b = bass.AP(
                            tensor=alpha.tensor, offset=alpha.offset,
                            ap=[list(alpha.ap[0]), list(alpha.ap[1]), [0, 16]])
                        kb = ci * KC + half * 16
                        nc.vector.tensor_mul(zb[:, kb:kb + 16, :],
                                             raw3[:, :, 0:16], alpha_b)

            # ================= phase C: attention =================
            with tc.tile_pool(name="eC", bufs=2) as ep, \
                 tc.tile_pool(name="oC", bufs=1) as op_, \
                 tc.tile_pool(name="psC", bufs=2, space="PSUM") as psC, \
                 tc.tile_pool(name="psO", bufs=1, space="PSUM") as psO:
                o_ps = psO.tile([128, D], F32, name="o_ps")
                for grp in range(8):
                    for h2 in range(2):
                        h = 2 * grp + h2
                        b, r = divmod(h, 4)
                        rs_ = slice(r * 32, (r + 1) * 32)
                        ps_s = psC.tile([128, 1024], F32, tag="sc")
                        for half in range(2):
                            hs = slice(half * 512, (half + 1) * 512)
                            nc.tensor.matmul(ps_s[:, hs], QT[b][rs_, :],
                                             KT[b][rs_, hs],
                                             start=True, stop=False,
                                             tile_position=(r * 32, 0))
                            nc.tensor.matmul(ps_s[:, hs], ident,
                                             zb[:, hs, h],
                                             start=False, stop=True)
                        e_sb = ep.tile([128, N], F32R, tag="e")
                        nc.scalar.activation(out=e_sb, in_=ps_s, func=AF.Exp,
                                             accum_out=rowsums[:, h:h + 1])
                        rec = ep.tile([128, 1], F32, tag="rec")
                        nc.vector.reciprocal(rec, rowsums[:, h:h + 1])
                        en = ep.tile([128, N], F32R, tag="en")
                        nc.vector.tensor_scalar_mul(en, e_sb, rec)
                        eT_ps = psC.tile([128, 1024], F32R, tag="sc")
                        for tt in range(8):
                            nc.tensor.transpose(eT_ps[:, tt * 128:(tt + 1) * 128],
                                                en[:, tt * 128:(tt + 1) * 128],
                                                ident)
                        eT = ep.tile([128, N], F32R, tag="eT")
                        nc.scalar.copy(eT, eT_ps)
                        for tt in range(8):
                            nc.tensor.matmul(
                                o_ps[:, h * 32:(h + 1) * 32],
                                eT[:, tt * 128:(tt + 1) * 128],
                                Vt[tt][:, h * 32:(h + 1) * 32],
                                start=(tt == 0), stop=(tt == 7))
                # og and final projection
                wo = [op_.tile([128, D], F32R, name=f"wo{g}") for g in range(4)]
                for g in range(4):
                    nc.sync.dma_start(out=wo[g], in_=wo_d[g * 128:(g + 1) * 128, :])
                og_nat = op_.tile([128, D], F32R, name="og_nat")
                nc.vector.tensor_mul(og_nat, o_ps, G_sb)
                ps_tr2 = psC.tile([128, D], F32R, tag="sc")
                for g in range(4):
                    nc.tensor.transpose(ps_tr2[:, g * 128:(g + 1) * 128],
                                        og_nat[:, g * 128:(g + 1) * 128], ident)
                og = [op_.tile([128, 128], F32R, name=f"og{g}") for g in range(4)]
                for g in range(4):
                    nc.scalar.copy(og[g], ps_tr2[:, g * 128:(g + 1) * 128])
                ps_out = psC.tile([128, 512], F32, tag="sc")
                for g in range(4):
                    nc.tensor.matmul(ps_out, og[g], wo[g],
                                     start=(g == 0), stop=(g == 3))
                out_sb = op_.tile([128, D], F32)
                nc.scalar.copy(out_sb, ps_out)
                nc.sync.dma_start(out=out_d[:, :], in_=out_sb)

    nc.compile()
    return nc


def _get_nc():
    global _CACHED
    if _CACHED is None:
        _CACHED = _build()
    return _CACHED


def _prepare_inputs(s, z, norm_s_w, norm_s_b, Wq, bq, Wk, Wv, Wg,
                    z_norm_w, z_norm_b, Wz, Wo):
    s2 = np.asarray(s, np.float32).reshape(N, D)
    z3 = np.asarray(z, np.float32).reshape(N, N, DZ)
    w_s = np.asarray(norm_s_w, np.float32)
    b_s = np.asarray(norm_s_b, np.float32)
    scale = np.float32(HD ** -0.5)
    Wq_f = (w_s[:, None] * np.asarray(Wq, np.float32)) * scale
    bq_f = (np.asarray(bq, np.float32) + b_s @ np.asarray(Wq, np.float32)) * scale
    Wk_f = w_s[:, None] * np.asarray(Wk, np.float32)
    bk_f = b_s @ np.asarray(Wk, np.float32)
    Wv_f = w_s[:, None] * np.asarray(Wv, np.float32)
    bv_f = b_s @ np.asarray(Wv, np.float32)
    Wg_f = w_s[:, None] * np.asarray(Wg, np.float32)
    bg_f = b_s @ np.asarray(Wg, np.float32)
    Wp = np.asarray(z_norm_w, np.float32)[:, None] * np.asarray(Wz, np.float32)
    S = Wp.sum(0)
    Wpp = Wp - np.ones((DZ, 1), np.float32) @ (S[None, :] / DZ)
    Wext = np.ascontiguousarray(
        np.concatenate([Wpp, np.ones((DZ, 1), np.float32),
                        np.zeros((DZ, 1), np.float32)], 1))
    ident = np.eye(128, dtype=np.float32)
    shared = {
        "s_full": s2, "Wq": np.ascontiguousarray(Wq_f),
        "Wk": np.ascontiguousarray(Wk_f), "Wv": np.ascontiguousarray(Wv_f),
        "Wg": np.ascontiguousarray(Wg_f),
        "Wo": np.ascontiguousarray(np.asarray(Wo, np.float32)),
        "bq": np.ascontiguousarray(bq_f), "bk": np.ascontiguousarray(bk_f),
        "bv": np.ascontiguousarray(bv_f), "bg": np.ascontiguousarray(bg_f),
        "Wext": Wext, "ident": ident,
    }
    in_maps = []
    for c in range(NC):
        qs = slice(c * NQ, (c + 1) * NQ)
        zTc = np.ascontiguousarray(z3[qs].transpose(2, 1, 0))
        m = dict(shared)
        m["s_q"] = np.ascontiguousarray(s2[qs])
        m["zT"] = zTc
        in_maps.append(m)
    return in_maps


def _run(in_maps, trace=False):
    nc = _get_nc()
    return run_bass_kernel_spmd(nc, in_maps, core_ids=list(range(NC)),
                                trace=trace)


def kernel(**inputs):
    in_maps = _prepare_inputs(**inputs)
    res = _run(in_maps, trace=False)
    out = np.concatenate([res.results[c]["out"] for c in range(NC)], 0)
    return out.reshape(B, N, D).astype(np.float32)
